# revision 17
# baseline (speedup 1.0000x reference)
"""AdaptiveConv2DMod Trainium2 kernel.

Data-parallel over batch b=8 across 8 NeuronCores; each core computes its
sample's modulated/demodulated weights and the groups=b conv (one group).

Per core:
  sel = softmax(embed @ adapt_w.T + adapt_b)            [4]
  mod = embed @ mod_w.T + mod_b                         [512]
  Wc  = sum_n sel_n * weights[n]                        [512,512,3,3]  (fp32)
  V   = Wc * (mod[i]+1)                                 (bf16, modulated)
  inv_norm[o] = rsqrt(clip(sum_{i,k,l} V^2, 1e-8))
  out[o,y,x]  = inv_norm[o] * sum_{i,ky,kx} V[o,i,ky,kx] * fmap[i,y+ky-1,x+kx-1]

Conv is an implicit GEMM: for each o-chunk(128) x spatial tile(8 rows x 64),
accumulate 36 matmuls (4 i-chunks x 9 taps) into one PSUM bank; the fmap
lives in SBUF as a zero-padded bf16 [128, 66, 66] per i-chunk so every tap
is a simple strided slice. Weights are combined in their native [o, i*9]
layout on the DVE (scalar_tensor_tensor FMA chain), then transposed to the
[i, o] layout the TensorE contraction needs via PE transposes.
"""

import sys

if "/opt/trn_rl_repo" not in sys.path:
    sys.path.insert(0, "/opt/trn_rl_repo")

import numpy as np

import concourse.bass as bass
import concourse.tile as tile
from concourse import bacc, mybir
from concourse.bass_utils import run_bass_kernel_spmd
from concourse.masks import make_identity

F32 = mybir.dt.float32
BF16 = mybir.dt.bfloat16

O, I, H, W, KS, NB = 512, 512, 64, 64, 3, 4
OC = O // 128   # o chunks
IC = I // 128   # i chunks
PW = W + 2      # padded width/height (66)
NT = H // 8     # spatial tiles (8 rows x 64 cols = 512)
EPS = 1e-8

_CACHED = {}


def _build():
    nc = bacc.Bacc("TRN2", target_bir_lowering=False, debug=False, num_devices=8)

    fmap = nc.dram_tensor("fmap", [I, H, W], F32, kind="ExternalInput").ap()
    embed = nc.dram_tensor("embed", [512], F32, kind="ExternalInput").ap()
    weights = nc.dram_tensor("weights", [NB, O, I, KS, KS], F32, kind="ExternalInput").ap()
    mod_w = nc.dram_tensor("mod_w", [512, 512], F32, kind="ExternalInput").ap()
    mod_b = nc.dram_tensor("mod_b", [512], F32, kind="ExternalInput").ap()
    adapt_w = nc.dram_tensor("adapt_w", [NB, 512], F32, kind="ExternalInput").ap()
    adapt_b = nc.dram_tensor("adapt_b", [NB], F32, kind="ExternalInput").ap()
    out = nc.dram_tensor("out", [O, H, W], F32, kind="ExternalOutput").ap()

    with tile.TileContext(nc) as tc:
        _emit(nc, tc, fmap, embed, weights, mod_w, mod_b, adapt_w, adapt_b, out)

    nc.compile()
    return nc


def _emit(nc, tc, fmap, embed, weights, mod_w, mod_b, adapt_w, adapt_b, out):
    import contextlib

    ctx = contextlib.ExitStack()
    with ctx:
        const = ctx.enter_context(tc.tile_pool(name="const", bufs=1))
        small = ctx.enter_context(tc.tile_pool(name="small", bufs=2))
        fstage_p = ctx.enter_context(tc.tile_pool(name="fstage", bufs=2))
        wbank_p = ctx.enter_context(tc.tile_pool(name="wbank", bufs=2))
        acc_p = ctx.enter_context(tc.tile_pool(name="acc", bufs=2))
        v_p = ctx.enter_context(tc.tile_pool(name="v", bufs=2))
        ob_p = ctx.enter_context(tc.tile_pool(name="ob", bufs=4))
        pt_p = ctx.enter_context(tc.tile_pool(name="pt", bufs=2, space="PSUM"))
        gr_p = ctx.enter_context(tc.tile_pool(name="gr", bufs=2, space="PSUM"))
        ps_p = ctx.enter_context(tc.tile_pool(name="ps", bufs=4, space="PSUM"))
        dram_p = ctx.enter_context(tc.tile_pool(name="dram", bufs=1, space="DRAM"))

        ident = const.tile([128, 128], BF16, tag="ident")
        make_identity(nc, ident)

        # ---------------- stage A: sel + mod + mscale ----------------
        embed_b = const.tile([128, 512], F32, tag="embed_b")
        nc.gpsimd.dma_start(
            out=embed_b,
            in_=bass.AP(tensor=embed.tensor, offset=embed.offset,
                        ap=[[0, 128], [1, 512]]),
        )

        # logits[n] = sum_k embed[k] * adapt_w[n, k] + adapt_b[n]
        aw = const.tile([NB, 512], F32, tag="aw")
        nc.gpsimd.dma_start(out=aw, in_=adapt_w[:, :])
        ab = const.tile([NB, 1], F32, tag="ab")
        nc.gpsimd.dma_start(out=ab, in_=adapt_b[:])
        junk4 = const.tile([NB, 512], F32, tag="junk4")
        logits = const.tile([NB, 1], F32, tag="logits")
        nc.vector.scalar_tensor_tensor(
            out=junk4, in0=aw, scalar=1.0, in1=embed_b[:NB, :],
            op0=mybir.AluOpType.bypass, op1=mybir.AluOpType.mult,
            accum_out=logits,
        )
        nc.vector.tensor_tensor(
            out=logits, in0=logits, in1=ab, op=mybir.AluOpType.add
        )

        # softmax over the 4 logits (flip partition->free, do it on 1 row;
        # logits are O(1) randn so no max-subtraction needed for fp32 exp)
        lg_f = const.tile([1, NB], F32, tag="lg_f")
        nc.gpsimd.dma_start(out=lg_f, in_=logits)
        ex = const.tile([1, NB], F32, tag="ex")
        nc.scalar.activation(out=ex, in_=lg_f,
                             func=mybir.ActivationFunctionType.Exp,
                             bias=0.0, scale=1.0)
        sm = const.tile([1, 1], F32, tag="sm")
        nc.vector.tensor_reduce(out=sm, in_=ex, axis=mybir.AxisListType.X,
                                op=mybir.AluOpType.add)
        rs = const.tile([1, 1], F32, tag="rs")
        nc.vector.reciprocal(out=rs, in_=sm)
        sel_f = const.tile([1, NB], F32, tag="sel_f")
        nc.vector.tensor_scalar_mul(out=sel_f, in0=ex, scalar1=rs)
        sel_d = dram_p.tile([NB], F32, tag="sel_d")
        nc.gpsimd.dma_start(out=sel_d, in_=sel_f)
        sel_b = const.tile([128, NB], F32, tag="sel_b")
        nc.gpsimd.dma_start(
            out=sel_b,
            in_=bass.AP(tensor=sel_d.tensor, offset=sel_d.offset,
                        ap=[[0, 128], [1, NB]]),
        )

        # mod[m] = sum_k embed[k] * mod_w[m, k]
        mod_t = const.tile([128, 4], F32, tag="mod_t")
        for c in range(4):
            mw = small.tile([128, 512], F32, tag="mw")
            nc.gpsimd.dma_start(out=mw, in_=mod_w[c * 128:(c + 1) * 128, :])
            junk = small.tile([128, 512], F32, tag="junk")
            nc.vector.scalar_tensor_tensor(
                out=junk, in0=mw, scalar=1.0, in1=embed_b,
                op0=mybir.AluOpType.bypass, op1=mybir.AluOpType.mult,
                accum_out=mod_t[:, c:c + 1],
            )

        # mscale = mod + mod_b + 1, laid out [128 part, 4] (i = c*128+p)
        modb_t = const.tile([128, 4], F32, tag="modb_t")
        nc.gpsimd.dma_start(
            out=modb_t,
            in_=bass.AP(tensor=mod_b.tensor, offset=mod_b.offset,
                        ap=[[1, 128], [128, 4]]),
        )
        msc = const.tile([128, 4], F32, tag="msc")
        nc.vector.scalar_tensor_tensor(
            out=msc, in0=mod_t, scalar=1.0, in1=modb_t,
            op0=mybir.AluOpType.add, op1=mybir.AluOpType.add,
        )
        ident_f = const.tile([128, 128], F32, tag="ident_f")
        make_identity(nc, ident_f)

        # ---------------- fmap: cast + pad ----------------
        fpad = []
        for c in range(IC):
            fp = const.tile([128, PW, PW], BF16, tag=f"fpad{c}")
            # zero only the borders (top/bottom rows, left/right columns)
            nc.gpsimd.memset(fp[:, 0, :], 0.0)
            nc.gpsimd.memset(fp[:, PW - 1, :], 0.0)
            nc.gpsimd.memset(fp[:, 1:PW - 1, 0:1], 0.0)
            nc.gpsimd.memset(fp[:, 1:PW - 1, PW - 1:PW], 0.0)
            fpad.append(fp)
        for c in range(IC):
            fst = fstage_p.tile([128, H, W], BF16, tag="fst")
            nc.gpsimd.dma_start(out=fst, in_=fmap[c * 128:(c + 1) * 128, :, :])
            nc.scalar.copy(out=fpad[c][:, 1:H + 1, 1:W + 1], in_=fst)

        # persistent transposed+modulated weights [i_chunk][128(i), 9, 512(o)]
        wct = [const.tile([128, 9, O], BF16, tag=f"wct{c}", name=f"wct{c}")
               for c in range(IC)]
        inv_norm = const.tile([128, OC], F32, tag="inv_norm")
        nt1 = const.tile([128, 1], F32, tag="nt1")
        junk_g = const.tile([128, 128], F32, tag="junk_g")

        for q in range(OC):
            # ---------------- weights pipeline for o-chunk q ----------------
            for c in range(IC):
                wb = [wbank_p.tile([128, 128, KS * KS], F32, tag=f"w{n}",
                                   name=f"w{n}")
                      for n in range(NB)]
                for n in range(NB):
                    nc.sync.dma_start(
                        out=wb[n],
                        in_=weights[n, q * 128:(q + 1) * 128,
                                    c * 128:(c + 1) * 128, :, :],
                    )
                acc = acc_p.tile([128, 128, KS * KS], F32, tag="acc")
                nc.vector.tensor_scalar_mul(out=acc, in0=wb[0],
                                            scalar1=sel_b[:, 0:1])
                for n in range(1, NB - 1):
                    nc.vector.scalar_tensor_tensor(
                        out=acc, in0=wb[n], scalar=sel_b[:, n:n + 1], in1=acc,
                        op0=mybir.AluOpType.mult, op1=mybir.AluOpType.add,
                    )
                # last bank: write tap-major bf16 (strided reads, dense write)
                v = v_p.tile([128, KS * KS, 128], BF16, tag="v")
                nc.vector.scalar_tensor_tensor(
                    out=v, in0=wb[NB - 1].rearrange("p i j -> p j i"),
                    scalar=sel_b[:, NB - 1:NB],
                    in1=acc.rearrange("p i j -> p j i"),
                    op0=mybir.AluOpType.mult, op1=mybir.AluOpType.add,
                )
                # transpose each tap [o,i]->[i,o]; modulate by mscale[i] on
                # the PSUM->SBUF copy (i is the partition dim there)
                for j in range(KS * KS):
                    pt = pt_p.tile([128, 128], BF16, tag="pt")
                    nc.tensor.transpose(pt, v[:, j, :], ident)
                    nc.scalar.activation(
                        out=wct[c][:, j, q * 128:(q + 1) * 128], in_=pt,
                        func=mybir.ActivationFunctionType.Copy,
                        scale=msc[:, c:c + 1],
                    )

            # demod: norm_sq[o] = diag(WcT_q.T @ WcT_q) via Gram matmul
            gr = gr_p.tile([128, 128], F32, tag="gr")
            for c in range(IC):
                for j in range(KS * KS):
                    lw = wct[c][:, j, q * 128:(q + 1) * 128]
                    nc.tensor.matmul(
                        gr, lw, lw,
                        start=(c == 0 and j == 0),
                        stop=(c == IC - 1 and j == KS * KS - 1),
                    )
            nc.vector.scalar_tensor_tensor(
                out=junk_g, in0=gr, scalar=1.0, in1=ident_f,
                op0=mybir.AluOpType.bypass, op1=mybir.AluOpType.mult,
                accum_out=nt1,
            )
            nc.vector.tensor_scalar_max(out=nt1, in0=nt1, scalar1=EPS)
            nc.scalar.sqrt(out=nt1, in_=nt1)
            nc.vector.reciprocal(out=inv_norm[:, q:q + 1], in_=nt1)

            # ---------------- conv for o-chunk q ----------------
            for t in range(NT):
                ps = ps_p.tile([128, 512], F32, tag="ps")
                first = True
                for c in range(IC):
                    for ky in range(KS):
                        for kx in range(KS):
                            j = ky * KS + kx
                            last = (c == IC - 1) and (j == KS * KS - 1)
                            nc.tensor.matmul(
                                ps,
                                wct[c][:, j, q * 128:(q + 1) * 128],
                                fpad[c][:, t * 8 + ky:t * 8 + ky + 8,
                                        kx:kx + W],
                                start=first, stop=last,
                            )
                            first = False
                ob = ob_p.tile([128, 8, W], F32, tag="ob")
                nc.scalar.activation(
                    out=ob, in_=ps.rearrange("p (a b) -> p a b", b=W),
                    func=mybir.ActivationFunctionType.Copy,
                    scale=inv_norm[:, q:q + 1],
                )
                nc.sync.dma_start(
                    out=out[q * 128:(q + 1) * 128, t * 8:(t + 1) * 8, :],
                    in_=ob,
                )


def _get_nc():
    if "nc" not in _CACHED:
        _CACHED["nc"] = _build()
    return _CACHED["nc"]


def _run(inputs, trace=False):
    nc = _get_nc()
    fmap = np.ascontiguousarray(inputs["fmap"], dtype=np.float32)
    embed = np.ascontiguousarray(inputs["embed"], dtype=np.float32)
    shared = {
        "weights": np.ascontiguousarray(inputs["weights"], dtype=np.float32),
        "mod_w": np.ascontiguousarray(inputs["mod_w"], dtype=np.float32),
        "mod_b": np.ascontiguousarray(inputs["mod_b"], dtype=np.float32),
        "adapt_w": np.ascontiguousarray(inputs["adapt_w"], dtype=np.float32),
        "adapt_b": np.ascontiguousarray(inputs["adapt_b"], dtype=np.float32),
    }
    b = fmap.shape[0]
    in_maps = [
        {"fmap": np.ascontiguousarray(fmap[c]),
         "embed": np.ascontiguousarray(embed[c]), **shared}
        for c in range(b)
    ]
    res = run_bass_kernel_spmd(nc, in_maps, core_ids=list(range(b)),
                               trace=trace)
    _CACHED["last_res"] = res
    outs = np.stack([res.results[c]["out"] for c in range(b)], axis=0)
    return outs.astype(np.float32), res.exec_time_ns


def kernel(**inputs):
    out, _ = _run(inputs, trace=False)
    return out


def kernel_traced(**inputs):
    return _run(inputs, trace=True)


# revision 22
# speedup vs baseline: 1.0220x; 1.0220x over previous
"""AdaptiveConv2DMod Trainium2 kernel.

Data-parallel over batch b=8 across 8 NeuronCores; each core computes its
sample's modulated/demodulated weights and the groups=b conv (one group).

Per core:
  sel = softmax(embed @ adapt_w.T + adapt_b)            [4]
  mod = embed @ mod_w.T + mod_b                         [512]
  Wc  = sum_n sel_n * weights[n]                        [512,512,3,3]  (fp32)
  V   = Wc * (mod[i]+1)                                 (bf16, modulated)
  inv_norm[o] = rsqrt(clip(sum_{i,k,l} V^2, 1e-8))
  out[o,y,x]  = inv_norm[o] * sum_{i,ky,kx} V[o,i,ky,kx] * fmap[i,y+ky-1,x+kx-1]

Conv is an implicit GEMM: for each o-chunk(128) x spatial tile(8 rows x 64),
accumulate 36 matmuls (4 i-chunks x 9 taps) into one PSUM bank; the fmap
lives in SBUF as a zero-padded bf16 [128, 66, 66] per i-chunk so every tap
is a simple strided slice. Weights are combined in their native [o, i*9]
layout on the DVE (scalar_tensor_tensor FMA chain), then transposed to the
[i, o] layout the TensorE contraction needs via PE transposes.
"""

import sys

if "/opt/trn_rl_repo" not in sys.path:
    sys.path.insert(0, "/opt/trn_rl_repo")

import numpy as np

import concourse.bass as bass
import concourse.tile as tile
from concourse import bacc, mybir
from concourse.bass_utils import run_bass_kernel_spmd
from concourse.masks import make_identity

F32 = mybir.dt.float32
BF16 = mybir.dt.bfloat16

O, I, H, W, KS, NB = 512, 512, 64, 64, 3, 4
OC = O // 128   # o chunks
IC = I // 128   # i chunks
PW = W + 2      # padded width/height (66)
NT = H // 8     # spatial tiles (8 rows x 64 cols = 512)
EPS = 1e-8

_CACHED = {}


def _build():
    nc = bacc.Bacc("TRN2", target_bir_lowering=False, debug=False, num_devices=8)

    fmap = nc.dram_tensor("fmap", [I, H, W], F32, kind="ExternalInput").ap()
    embed = nc.dram_tensor("embed", [512], F32, kind="ExternalInput").ap()
    weights = nc.dram_tensor("weights", [NB, O, I, KS, KS], F32, kind="ExternalInput").ap()
    mod_w = nc.dram_tensor("mod_w", [512, 512], F32, kind="ExternalInput").ap()
    mod_b = nc.dram_tensor("mod_b", [512], F32, kind="ExternalInput").ap()
    adapt_w = nc.dram_tensor("adapt_w", [NB, 512], F32, kind="ExternalInput").ap()
    adapt_b = nc.dram_tensor("adapt_b", [NB], F32, kind="ExternalInput").ap()
    out = nc.dram_tensor("out", [O, H, W], F32, kind="ExternalOutput").ap()

    with tile.TileContext(nc) as tc:
        _emit(nc, tc, fmap, embed, weights, mod_w, mod_b, adapt_w, adapt_b, out)

    nc.compile()
    return nc


def _emit(nc, tc, fmap, embed, weights, mod_w, mod_b, adapt_w, adapt_b, out):
    import contextlib

    ctx = contextlib.ExitStack()
    with ctx:
        const = ctx.enter_context(tc.tile_pool(name="const", bufs=1))
        small = ctx.enter_context(tc.tile_pool(name="small", bufs=2))
        fstage_p = ctx.enter_context(tc.tile_pool(name="fstage", bufs=2))
        wbank_p = ctx.enter_context(tc.tile_pool(name="wbank", bufs=2))
        acc_p = ctx.enter_context(tc.tile_pool(name="acc", bufs=2))
        v_p = ctx.enter_context(tc.tile_pool(name="v", bufs=2))
        ob_p = ctx.enter_context(tc.tile_pool(name="ob", bufs=4))
        pt_p = ctx.enter_context(tc.tile_pool(name="pt", bufs=2, space="PSUM"))
        gr_p = ctx.enter_context(tc.tile_pool(name="gr", bufs=2, space="PSUM"))
        ps_p = ctx.enter_context(tc.tile_pool(name="ps", bufs=4, space="PSUM"))
        dram_p = ctx.enter_context(tc.tile_pool(name="dram", bufs=1, space="DRAM"))

        ident = const.tile([128, 128], BF16, tag="ident")
        make_identity(nc, ident)

        # ---------------- stage A: sel + mod + mscale ----------------
        embed_b = const.tile([128, 512], F32, tag="embed_b")
        nc.gpsimd.dma_start(
            out=embed_b,
            in_=bass.AP(tensor=embed.tensor, offset=embed.offset,
                        ap=[[0, 128], [1, 512]]),
        )

        # logits[n] = sum_k embed[k] * adapt_w[n, k] + adapt_b[n]
        aw = const.tile([NB, 512], F32, tag="aw")
        nc.gpsimd.dma_start(out=aw, in_=adapt_w[:, :])
        ab = const.tile([NB, 1], F32, tag="ab")
        nc.gpsimd.dma_start(out=ab, in_=adapt_b[:])
        junk4 = const.tile([NB, 512], F32, tag="junk4")
        logits = const.tile([NB, 1], F32, tag="logits")
        nc.vector.scalar_tensor_tensor(
            out=junk4, in0=aw, scalar=1.0, in1=embed_b[:NB, :],
            op0=mybir.AluOpType.bypass, op1=mybir.AluOpType.mult,
            accum_out=logits,
        )
        nc.vector.tensor_tensor(
            out=logits, in0=logits, in1=ab, op=mybir.AluOpType.add
        )

        # softmax over the 4 logits (flip partition->free, do it on 1 row;
        # logits are O(1) randn so no max-subtraction needed for fp32 exp).
        # The flip/broadcast DMAs ride the Scalar engine's HWDGE so they
        # interleave with the Exp instead of stalling the bulk-load queues.
        lg_f = const.tile([1, NB], F32, tag="lg_f")
        nc.scalar.dma_start(out=lg_f, in_=logits)
        ex = const.tile([1, NB], F32, tag="ex")
        nc.scalar.activation(out=ex, in_=lg_f,
                             func=mybir.ActivationFunctionType.Exp,
                             bias=0.0, scale=1.0)
        sm = const.tile([1, 1], F32, tag="sm")
        nc.vector.tensor_reduce(out=sm, in_=ex, axis=mybir.AxisListType.X,
                                op=mybir.AluOpType.add)
        rs = const.tile([1, 1], F32, tag="rs")
        nc.vector.reciprocal(out=rs, in_=sm)
        sel_f = const.tile([1, NB], F32, tag="sel_f")
        nc.vector.tensor_scalar_mul(out=sel_f, in0=ex, scalar1=rs)
        sel_d = dram_p.tile([NB], F32, tag="sel_d")
        nc.scalar.dma_start(out=sel_d, in_=sel_f)
        sel_b = const.tile([128, NB], F32, tag="sel_b")
        nc.scalar.dma_start(
            out=sel_b,
            in_=bass.AP(tensor=sel_d.tensor, offset=sel_d.offset,
                        ap=[[0, 128], [1, NB]]),
        )

        # mod[m] = sum_k embed[k] * mod_w[m, k]
        mod_t = const.tile([128, 4], F32, tag="mod_t")
        for c in range(4):
            mw = small.tile([128, 512], F32, tag="mw")
            nc.sync.dma_start(out=mw, in_=mod_w[c * 128:(c + 1) * 128, :])
            junk = small.tile([128, 512], F32, tag="junk")
            nc.vector.scalar_tensor_tensor(
                out=junk, in0=mw, scalar=1.0, in1=embed_b,
                op0=mybir.AluOpType.bypass, op1=mybir.AluOpType.mult,
                accum_out=mod_t[:, c:c + 1],
            )

        # mscale = mod + mod_b + 1, laid out [128 part, 4] (i = c*128+p)
        modb_t = const.tile([128, 4], F32, tag="modb_t")
        nc.sync.dma_start(
            out=modb_t,
            in_=bass.AP(tensor=mod_b.tensor, offset=mod_b.offset,
                        ap=[[1, 128], [128, 4]]),
        )
        msc = const.tile([128, 4], F32, tag="msc")
        nc.vector.scalar_tensor_tensor(
            out=msc, in0=mod_t, scalar=1.0, in1=modb_t,
            op0=mybir.AluOpType.add, op1=mybir.AluOpType.add,
        )
        ident_f = const.tile([128, 128], F32, tag="ident_f")
        make_identity(nc, ident_f)

        # ---------------- fmap: cast + pad ----------------
        fpad = []
        for c in range(IC):
            fp = const.tile([128, PW, PW], BF16, tag=f"fpad{c}")
            # zero only the borders (top/bottom rows, left/right columns)
            nc.gpsimd.memset(fp[:, 0, :], 0.0)
            nc.gpsimd.memset(fp[:, PW - 1, :], 0.0)
            nc.gpsimd.memset(fp[:, 1:PW - 1, 0:1], 0.0)
            nc.gpsimd.memset(fp[:, 1:PW - 1, PW - 1:PW], 0.0)
            fpad.append(fp)
        for c in range(IC):
            fst = fstage_p.tile([128, H, W], BF16, tag="fst")
            nc.gpsimd.dma_start(out=fst, in_=fmap[c * 128:(c + 1) * 128, :, :])
            nc.scalar.copy(out=fpad[c][:, 1:H + 1, 1:W + 1], in_=fst)

        # persistent transposed+modulated weights [i_chunk][128(i), 9, 512(o)]
        wct = [const.tile([128, 9, O], BF16, tag=f"wct{c}", name=f"wct{c}")
               for c in range(IC)]
        inv_norm = const.tile([128, OC], F32, tag="inv_norm")
        nt1 = const.tile([128, 1], F32, tag="nt1")
        junk_g = const.tile([128, 128], F32, tag="junk_g")

        for q in range(OC):
            # ---------------- weights pipeline for o-chunk q ----------------
            for c in range(IC):
                wb = [wbank_p.tile([128, 128, KS * KS], F32, tag=f"w{n}",
                                   name=f"w{n}")
                      for n in range(NB)]
                for n in range(NB):
                    nc.sync.dma_start(
                        out=wb[n],
                        in_=weights[n, q * 128:(q + 1) * 128,
                                    c * 128:(c + 1) * 128, :, :],
                    )
                acc = acc_p.tile([128, 128, KS * KS], F32, tag="acc")
                nc.vector.tensor_scalar_mul(out=acc, in0=wb[0],
                                            scalar1=sel_b[:, 0:1])
                for n in range(1, NB - 1):
                    nc.vector.scalar_tensor_tensor(
                        out=acc, in0=wb[n], scalar=sel_b[:, n:n + 1], in1=acc,
                        op0=mybir.AluOpType.mult, op1=mybir.AluOpType.add,
                    )
                # last bank: write tap-major bf16 (strided reads, dense write)
                v = v_p.tile([128, KS * KS, 128], BF16, tag="v")
                nc.vector.scalar_tensor_tensor(
                    out=v, in0=wb[NB - 1].rearrange("p i j -> p j i"),
                    scalar=sel_b[:, NB - 1:NB],
                    in1=acc.rearrange("p i j -> p j i"),
                    op0=mybir.AluOpType.mult, op1=mybir.AluOpType.add,
                )
                # transpose each tap [o,i]->[i,o]; modulate by mscale[i] on
                # the PSUM->SBUF copy (i is the partition dim there)
                for j in range(KS * KS):
                    pt = pt_p.tile([128, 128], BF16, tag="pt")
                    nc.tensor.transpose(pt, v[:, j, :], ident)
                    nc.scalar.activation(
                        out=wct[c][:, j, q * 128:(q + 1) * 128], in_=pt,
                        func=mybir.ActivationFunctionType.Copy,
                        scale=msc[:, c:c + 1],
                    )

            # demod: norm_sq[o] = diag(WcT_q.T @ WcT_q) via Gram matmul
            gr = gr_p.tile([128, 128], F32, tag="gr")
            for c in range(IC):
                for j in range(KS * KS):
                    lw = wct[c][:, j, q * 128:(q + 1) * 128]
                    nc.tensor.matmul(
                        gr, lw, lw,
                        start=(c == 0 and j == 0),
                        stop=(c == IC - 1 and j == KS * KS - 1),
                    )
            nc.vector.scalar_tensor_tensor(
                out=junk_g, in0=gr, scalar=1.0, in1=ident_f,
                op0=mybir.AluOpType.bypass, op1=mybir.AluOpType.mult,
                accum_out=nt1,
            )
            nc.vector.tensor_scalar_max(out=nt1, in0=nt1, scalar1=EPS)
            nc.scalar.sqrt(out=nt1, in_=nt1)
            nc.vector.reciprocal(out=inv_norm[:, q:q + 1], in_=nt1)

            # ---------------- conv for o-chunk q ----------------
            for t in range(NT):
                ps = ps_p.tile([128, 512], F32, tag="ps")
                first = True
                for c in range(IC):
                    for ky in range(KS):
                        for kx in range(KS):
                            j = ky * KS + kx
                            last = (c == IC - 1) and (j == KS * KS - 1)
                            nc.tensor.matmul(
                                ps,
                                wct[c][:, j, q * 128:(q + 1) * 128],
                                fpad[c][:, t * 8 + ky:t * 8 + ky + 8,
                                        kx:kx + W],
                                start=first, stop=last,
                            )
                            first = False
                ob = ob_p.tile([128, 8, W], F32, tag="ob")
                nc.scalar.activation(
                    out=ob, in_=ps.rearrange("p (a b) -> p a b", b=W),
                    func=mybir.ActivationFunctionType.Copy,
                    scale=inv_norm[:, q:q + 1],
                )
                nc.sync.dma_start(
                    out=out[q * 128:(q + 1) * 128, t * 8:(t + 1) * 8, :],
                    in_=ob,
                )


def _get_nc():
    if "nc" not in _CACHED:
        _CACHED["nc"] = _build()
    return _CACHED["nc"]


def _run(inputs, trace=False):
    nc = _get_nc()
    fmap = np.ascontiguousarray(inputs["fmap"], dtype=np.float32)
    embed = np.ascontiguousarray(inputs["embed"], dtype=np.float32)
    shared = {
        "weights": np.ascontiguousarray(inputs["weights"], dtype=np.float32),
        "mod_w": np.ascontiguousarray(inputs["mod_w"], dtype=np.float32),
        "mod_b": np.ascontiguousarray(inputs["mod_b"], dtype=np.float32),
        "adapt_w": np.ascontiguousarray(inputs["adapt_w"], dtype=np.float32),
        "adapt_b": np.ascontiguousarray(inputs["adapt_b"], dtype=np.float32),
    }
    b = fmap.shape[0]
    in_maps = [
        {"fmap": np.ascontiguousarray(fmap[c]),
         "embed": np.ascontiguousarray(embed[c]), **shared}
        for c in range(b)
    ]
    res = run_bass_kernel_spmd(nc, in_maps, core_ids=list(range(b)),
                               trace=trace)
    _CACHED["last_res"] = res
    outs = np.stack([res.results[c]["out"] for c in range(b)], axis=0)
    return outs.astype(np.float32), res.exec_time_ns


def kernel(**inputs):
    out, _ = _run(inputs, trace=False)
    return out


def kernel_traced(**inputs):
    return _run(inputs, trace=True)
